# revision 7
# baseline (speedup 1.0000x reference)
"""YOLOv1 loss (nn_LossModul_16277926052544) on 8 TRN2 NeuronCores.

Pure data parallel: batch 8192 -> 8 shards of 1024. Each core computes a
partial sum of the loss over its shard; host sums the 8x128 partials.

Math restructuring vs the reference (validated to 5.5e-07 rel err in numpy):
  * IoU is translation invariant -> grid offsets cancel; overlap length per
    axis is min(pw, tw, (pw+tw)/2 - |c|) clamped to >= 0, c = (px-tx)/S.
  * resp = iou1 > iou2 <=> inter1*den2 > inter2*den1 (dens > 0).
  * Every loss term is a masked square; weights fold into the masks
    (sqrt(5)*mo for xy, 5*mo pre-sqrt for wh, sqrt(.5)*(1-mo) for noobj), so
    the Scalar engine's Square+accum reduces everything.
"""
import sys

for _p in ("/opt/trn_rl_repo",):
    if _p not in sys.path:
        sys.path.insert(0, _p)

import numpy as np
from contextlib import ExitStack

import concourse.bass as bass  # noqa: F401  (registers engines)
from concourse import bacc, mybir
from concourse import bass_utils
import concourse.tile as tile

N_CORES = 8
BATCH = 8192
S = 7
C = 30
P = 128
CELLS_PER_CORE = (BATCH // N_CORES) * S * S   # 50176
F_TOTAL = CELLS_PER_CORE // P                 # 392
T_TILES = 2
F = F_TOTAL // T_TILES                        # 196
R = 1.0 / S
EPS5 = 5e-6                                   # 5 * EPS (lambda folded)
SQRT5 = float(np.sqrt(5.0))
SQH = float(np.sqrt(0.5))

f32 = mybir.dt.float32
Alu = mybir.AluOpType
Act = mybir.ActivationFunctionType

_CACHE = {}


def _build_body(tc, ctx, pred, targ, out_ap):
    nc = tc.nc
    inpool = ctx.enter_context(tc.tile_pool(name="in", bufs=2))
    wk = ctx.enter_context(tc.tile_pool(name="wk", bufs=2))
    stp = ctx.enter_context(tc.tile_pool(name="st", bufs=1))
    stats = stp.tile([P, 2 * T_TILES], f32)
    eps5c = stp.tile([P, 1], f32)               # bias const for Sqrt
    nc.gpsimd.memset(eps5c[:], EPS5)

    for t in range(T_TILES):
        ptile = inpool.tile([P, F * C], f32, tag="p")
        nc.sync.dma_start(ptile[:], pred[:, t].rearrange("p f c -> p (f c)"))
        ttile = inpool.tile([P, F * C], f32, tag="t")
        nc.sync.dma_start(ttile[:], targ[:, t].rearrange("p f c -> p (f c)"))

        pv = ptile[:].rearrange("p (f c) -> p f c", c=C)   # [P,F,30]
        tv = ttile[:].rearrange("p (f c) -> p f c", c=C)
        pb = pv[:, :, 0:10].rearrange("p f (b c) -> p f b c", b=2)  # [P,F,2,5]

        def b2(ap_pf1, k=2):
            # [P,F] (or [P,F,1]) -> broadcast over a trailing axis of size k
            a = ap_pf1
            if a.ndim == 2:
                a = a.unsqueeze(2)
            return a.broadcast_to([P, F, k])

        # ---------- geometry ----------
        dxy = wk.tile([P, F, 2, 2], f32, tag="dxy")   # [f, box, xy] raw p-t
        nc.vector.tensor_sub(
            dxy[:], pb[:, :, :, 0:2],
            tv[:, :, 0:2].unsqueeze(2).broadcast_to([P, F, 2, 2]))
        flat = lambda a: a.rearrange("p f b c -> p (f b c)")
        sth = wk.tile([P, F, 2], f32, tag="sth")      # 0.5*twh
        nc.vector.tensor_scalar_mul(sth[:], tv[:, :, 2:4], 0.5)
        s = wk.tile([P, F, 2, 2], f32, tag="s")       # 0.5*pwh + 0.5*twh
        for b in range(2):                             # STT is <=3D
            nc.vector.scalar_tensor_tensor(
                s[:, :, b, :], pb[:, :, b, 2:4], 0.5, sth[:],
                op0=Alu.mult, op1=Alu.add)
        absd = wk.tile([P, F, 2, 2], f32, tag="absd")  # |dxy|
        nc.scalar.activation(flat(absd[:]), flat(dxy[:]), Act.Abs)
        m = wk.tile([P, F, 2, 2], f32, tag="m")       # s - R*|dxy|
        nc.vector.scalar_tensor_tensor(
            flat(m[:]), flat(absd[:]), -R, flat(s[:]),
            op0=Alu.mult, op1=Alu.add)
        minwh = wk.tile([P, F, 2, 2], f32, tag="minwh")
        nc.vector.tensor_tensor(
            minwh[:], pb[:, :, :, 2:4],
            tv[:, :, 2:4].unsqueeze(2).broadcast_to([P, F, 2, 2]), op=Alu.min)
        ln = wk.tile([P, F, 2, 2], f32, tag="ln")     # overlap lengths
        nc.vector.scalar_tensor_tensor(
            flat(ln[:]), flat(m[:]), 0.0, flat(minwh[:]),
            op0=Alu.max, op1=Alu.min)

        ID = wk.tile([P, F, 4], f32, tag="ID")        # [I1,I2,D1,D2]
        nc.vector.tensor_mul(ID[:, :, 0:2], ln[:, :, :, 0], ln[:, :, :, 1])
        nc.vector.tensor_mul(ID[:, :, 2:4], pb[:, :, :, 2], pb[:, :, :, 3])
        tarea = wk.tile([P, F, 1], f32, tag="tarea")
        nc.vector.tensor_mul(tarea[:], tv[:, :, 2:3], tv[:, :, 3:4])
        nc.vector.tensor_sub(ID[:, :, 2:4], ID[:, :, 2:4], ID[:, :, 0:2])
        nc.vector.tensor_add(ID[:, :, 2:4], ID[:, :, 2:4], b2(tarea[:]))

        g = wk.tile([P, F, 2], f32, tag="g")
        nc.vector.tensor_mul(g[:, :, 0:1], ID[:, :, 0:1], ID[:, :, 3:4])
        nc.vector.tensor_mul(g[:, :, 1:2], ID[:, :, 1:2], ID[:, :, 2:3])
        resp = wk.tile([P, F, 1], mybir.dt.uint32, tag="resp")  # 1 -> box1
        nc.vector.tensor_tensor(resp[:], g[:, :, 0:1], g[:, :, 1:2], op=Alu.is_gt)

        # ---------- selects (box2 copied, box1 predicated over it) ----------
        sel = wk.tile([P, F, 8], f32, tag="sel")      # dx dy w h c I D pad
        nc.scalar.copy(sel[:, :, 0:2], dxy[:, :, 1, :])
        nc.scalar.copy(sel[:, :, 2:5], pv[:, :, 7:10])
        nc.scalar.copy(sel[:, :, 5:6], ID[:, :, 1:2])
        nc.scalar.copy(sel[:, :, 6:7], ID[:, :, 3:4])
        nc.vector.copy_predicated(sel[:, :, 0:2], b2(resp[:]), dxy[:, :, 0, :])
        nc.vector.copy_predicated(sel[:, :, 2:5], b2(resp[:], 3), pv[:, :, 2:5])
        nc.vector.copy_predicated(sel[:, :, 5:6], resp[:], ID[:, :, 0:1])
        nc.vector.copy_predicated(sel[:, :, 6:7], resp[:], ID[:, :, 2:3])

        rcp = wk.tile([P, F, 1], f32, tag="rcp")
        nc.vector.reciprocal_approx_fast(rcp[:], sel[:, :, 6:7])
        iou = wk.tile([P, F, 1], f32, tag="iou")
        nc.vector.tensor_mul(iou[:], sel[:, :, 5:6], rcp[:])
        nc.vector.scalar_tensor_tensor(                # c_sel - iou
            sel[:, :, 4:5], iou[:], -1.0, sel[:, :, 4:5],
            op0=Alu.mult, op1=Alu.add)

        # ---------- masks ----------
        mo = wk.tile([P, F, 1], f32, tag="mo")
        nc.vector.tensor_single_scalar(mo[:], tv[:, :, 4:5], 0.0, op=Alu.is_gt)
        mo5 = wk.tile([P, F, 1], f32, tag="mo5")
        nc.scalar.mul(mo5[:], mo[:], SQRT5)
        mo25 = wk.tile([P, F, 1], f32, tag="mo25")
        nc.scalar.mul(mo25[:], mo[:], 5.0)
        mnh = wk.tile([P, F, 1], f32, tag="mnh")      # sqrt(.5)*(1-mo)
        nc.vector.tensor_scalar(mnh[:], mo[:], -SQH, SQH,
                                op0=Alu.mult, op1=Alu.add)

        nc.vector.tensor_mul(sel[:, :, 0:2], sel[:, :, 0:2], b2(mo5[:]))
        nc.vector.tensor_mul(sel[:, :, 2:4], sel[:, :, 2:4], b2(mo25[:]))
        nc.vector.tensor_mul(sel[:, :, 4:5], sel[:, :, 4:5], mo[:])
        mtwh = wk.tile([P, F, 2], f32, tag="mtwh")
        nc.gpsimd.tensor_mul(mtwh[:], tv[:, :, 2:4], b2(mo25[:]))

        nc.scalar.activation(sel[:, :, 2:4], sel[:, :, 2:4], Act.Sqrt,
                             bias=eps5c[:])
        st = wk.tile([P, F, 2], f32, tag="stw")
        nc.scalar.activation(st[:], mtwh[:], Act.Sqrt, bias=eps5c[:])
        nc.vector.tensor_sub(sel[:, :, 2:4], sel[:, :, 2:4], st[:])

        # noobj into dead I/D slots
        nc.vector.tensor_mul(sel[:, :, 5:7], pb[:, :, :, 4], b2(mnh[:]))

        # ---------- cls (gpsimd) ----------
        mcls = wk.tile([P, F, 20], f32, tag="mcls")
        nc.gpsimd.tensor_sub(mcls[:], pv[:, :, 10:30], tv[:, :, 10:30])
        nc.gpsimd.tensor_mul(mcls[:], mcls[:], b2(mo[:], 20))

        # ---------- square-accumulate ----------
        nc.scalar.activation(sel[:, :, 0:7], sel[:, :, 0:7], Act.Square,
                             accum_out=stats[:, 2 * t:2 * t + 1])
        nc.scalar.activation(mcls[:], mcls[:], Act.Square,
                             accum_out=stats[:, 2 * t + 1:2 * t + 2])

    total = stp.tile([P, 1], f32)
    nc.vector.tensor_reduce(total[:], stats[:], axis=mybir.AxisListType.X,
                            op=Alu.add)
    nc.sync.dma_start(out_ap, total[:])


def _build():
    if "nc" in _CACHE:
        return _CACHE["nc"]
    nc = bacc.Bacc("TRN2", target_bir_lowering=False, debug=False)
    pred = nc.dram_tensor("predicts", [P, T_TILES, F, C], f32,
                          kind="ExternalInput")
    targ = nc.dram_tensor("targets", [P, T_TILES, F, C], f32,
                          kind="ExternalInput")
    out = nc.dram_tensor("out", [P, 1], f32, kind="ExternalOutput")
    with tile.TileContext(nc) as tc, ExitStack() as ctx:
        _build_body(tc, ctx, pred.ap(), targ.ap(), out.ap())
    nc.compile()
    _CACHE["nc"] = nc
    return nc


def _shard(x):
    # [8192,7,7,30] -> per-core [P, T_TILES, F, C], row-major cell split
    x = np.ascontiguousarray(x, dtype=np.float32)
    return [x[i * (BATCH // N_CORES):(i + 1) * (BATCH // N_CORES)]
            .reshape(P, T_TILES, F, C) for i in range(N_CORES)]


def run(predicts, targets, trace=False, **trace_kwargs):
    nc = _build()
    pshards = _shard(predicts)
    tshards = _shard(targets)
    in_maps = [{"predicts": pshards[i], "targets": tshards[i]}
               for i in range(N_CORES)]
    res = bass_utils.run_bass_kernel_spmd(
        nc, in_maps, core_ids=list(range(N_CORES)), trace=trace,
        **trace_kwargs)
    partial = np.zeros((), dtype=np.float64)
    for r in res.results:
        partial += np.asarray(r["out"], dtype=np.float64).sum()
    return np.float32(partial), res


def kernel(predicts, targets):
    out, _ = run(predicts, targets, trace=False)
    return out


# revision 9
# speedup vs baseline: 1.1352x; 1.1352x over previous
"""YOLOv1 loss (nn_LossModul_16277926052544) on 8 TRN2 NeuronCores.

Pure data parallel: batch 8192 -> 8 shards of 1024. Each core computes a
partial sum of the loss over its shard; host sums the 8x128 partials.

Math restructuring vs the reference (validated to 5.5e-07 rel err in numpy):
  * IoU is translation invariant -> grid offsets cancel; overlap length per
    axis is min(pw, tw, (pw+tw)/2 - |c|) clamped to >= 0, c = (px-tx)/S.
  * resp = iou1 > iou2 <=> inter1*den2 > inter2*den1 (dens > 0).
  * Every loss term is a masked square; weights fold into the masks
    (sqrt(5)*mo for xy, 5*mo pre-sqrt for wh, sqrt(.5)*(1-mo) for noobj), so
    the Scalar engine's Square+accum reduces everything.

Layout: channel-planar SBUF tiles ([P, C, F], inner F contiguous) -- the DVE
pays a per-AP-row bubble, so short strided inner dims (interleaved channels)
are ~8x slower than planar streams. The host pre-transposes each shard to
[P, T, C, F]; same byte count over DMA.
"""
import sys

for _p in ("/opt/trn_rl_repo",):
    if _p not in sys.path:
        sys.path.insert(0, _p)

import numpy as np
from contextlib import ExitStack

import concourse.bass as bass  # noqa: F401  (registers engines)
from concourse import bacc, mybir
from concourse import bass_utils
import concourse.tile as tile

N_CORES = 8
BATCH = 8192
S = 7
C = 30
P = 128
CELLS_PER_CORE = (BATCH // N_CORES) * S * S   # 50176
F_TOTAL = CELLS_PER_CORE // P                 # 392
T_TILES = 2
F = F_TOTAL // T_TILES                        # 196
R = 1.0 / S
EPS5 = 5e-6                                   # 5 * EPS (lambda folded)
SQRT5 = float(np.sqrt(5.0))
SQH = float(np.sqrt(0.5))

f32 = mybir.dt.float32
bf16 = mybir.dt.bfloat16
u32 = mybir.dt.uint32
Alu = mybir.AluOpType
Act = mybir.ActivationFunctionType

_CACHE = {}


def _build_body(tc, ctx, pred, targ, out_ap):
    nc = tc.nc
    inpool = ctx.enter_context(tc.tile_pool(name="in", bufs=2))
    wk = ctx.enter_context(tc.tile_pool(name="wk", bufs=2))
    stp = ctx.enter_context(tc.tile_pool(name="st", bufs=1))
    stats = stp.tile([P, 2 * T_TILES], f32)
    eps5c = stp.tile([P, 1], f32)               # bias const for Sqrt
    nc.gpsimd.memset(eps5c[:], EPS5)

    for t in range(T_TILES):
        ptile = inpool.tile([P, C * F], f32, tag="p")
        nc.sync.dma_start(ptile[:], pred[:, t].rearrange("p c f -> p (c f)"))
        ttile = inpool.tile([P, C * F], f32, tag="t")
        nc.sync.dma_start(ttile[:], targ[:, t].rearrange("p c f -> p (c f)"))

        pv = ptile[:].rearrange("p (c f) -> p c f", c=C)   # [P,30,F]
        tv = ttile[:].rearrange("p (c f) -> p c f", c=C)
        pb = pv[:, 0:10].rearrange("p (b c) f -> p b c f", b=2)  # [P,2,5,F]

        def bc(ap_pf, k):
            # [P,F] (or [P,1,F]) -> broadcast over a middle axis of size k
            a = ap_pf
            if a.ndim == 2:
                a = a.unsqueeze(1)
            return a.broadcast_to([P, k, F])

        flat = lambda a: a.rearrange("p b c f -> p (b c f)")

        # ---------- geometry ----------
        dxy = wk.tile([P, 2, 2, F], f32, tag="dxy")   # [box, xy, f] raw p-t
        nc.gpsimd.tensor_sub(
            dxy[:], pb[:, :, 0:2, :],
            tv[:, 0:2, :].unsqueeze(1).broadcast_to([P, 2, 2, F]))
        sth = wk.tile([P, 2, F], f32, tag="sth")      # 0.5*twh
        nc.vector.tensor_scalar_mul(sth[:], tv[:, 2:4, :], 0.5)
        s = wk.tile([P, 2, 2, F], f32, tag="s")       # 0.5*pwh + 0.5*twh
        for b in range(2):                             # STT is <=3D
            nc.vector.scalar_tensor_tensor(
                s[:, b], pb[:, b, 2:4, :], 0.5, sth[:],
                op0=Alu.mult, op1=Alu.add)
        absd = wk.tile([P, 2, 2, F], f32, tag="absd")  # |dxy|
        nc.scalar.activation(flat(absd[:]), flat(dxy[:]), Act.Abs)
        m = wk.tile([P, 2, 2, F], f32, tag="m")       # s - R*|dxy|
        nc.vector.scalar_tensor_tensor(
            flat(m[:]), flat(absd[:]), -R, flat(s[:]),
            op0=Alu.mult, op1=Alu.add)
        minwh = wk.tile([P, 2, 2, F], f32, tag="minwh")
        nc.vector.tensor_tensor(
            minwh[:], pb[:, :, 2:4, :],
            tv[:, 2:4, :].unsqueeze(1).broadcast_to([P, 2, 2, F]), op=Alu.min)
        ln = wk.tile([P, 2, 2, F], f32, tag="ln")     # overlap lengths
        nc.vector.scalar_tensor_tensor(
            flat(ln[:]), flat(m[:]), 0.0, flat(minwh[:]),
            op0=Alu.max, op1=Alu.min)

        ID = wk.tile([P, 4, F], f32, tag="ID")        # [I1,I2,D1,D2]
        nc.vector.tensor_mul(ID[:, 0:2, :], ln[:, :, 0, :], ln[:, :, 1, :])
        nc.vector.tensor_mul(ID[:, 2:4, :], pv[:, 2:8:5, :], pv[:, 3:9:5, :])
        tarea = wk.tile([P, 1, F], f32, tag="tarea")
        nc.vector.tensor_mul(tarea[:], tv[:, 2:3, :], tv[:, 3:4, :])
        nc.gpsimd.tensor_sub(ID[:, 2:4, :], ID[:, 2:4, :], ID[:, 0:2, :])
        nc.gpsimd.tensor_add(ID[:, 2:4, :], ID[:, 2:4, :], bc(tarea[:], 2))

        g = wk.tile([P, 2, F], f32, tag="g")
        nc.vector.tensor_mul(g[:, 0, :], ID[:, 0, :], ID[:, 3, :])
        nc.vector.tensor_mul(g[:, 1, :], ID[:, 1, :], ID[:, 2, :])
        resp = wk.tile([P, F], u32, tag="resp")       # 1 -> box1
        nc.vector.tensor_tensor(resp[:], g[:, 0, :], g[:, 1, :], op=Alu.is_gt)

        # ---------- selects (box2 copied, box1 predicated over it) ----------
        sel = wk.tile([P, 8, F], f32, tag="sel")      # dx dy w h c I D pad
        nc.scalar.copy(sel[:, 0:2, :], dxy[:, 1, :, :])
        nc.scalar.copy(sel[:, 2:5, :], pv[:, 7:10, :])
        nc.scalar.copy(sel[:, 5:7, :], ID[:, 1:4:2, :])
        nc.vector.copy_predicated(sel[:, 0:2, :], bc(resp[:], 2), dxy[:, 0, :, :])
        nc.vector.copy_predicated(sel[:, 2:5, :], bc(resp[:], 3), pv[:, 2:5, :])
        nc.vector.copy_predicated(sel[:, 5:7, :], bc(resp[:], 2), ID[:, 0:3:2, :])

        rcp = wk.tile([P, F], f32, tag="rcp")
        nc.vector.reciprocal_approx_fast(rcp[:], sel[:, 6, :])
        iou = wk.tile([P, F], f32, tag="iou")
        nc.vector.tensor_mul(iou[:], sel[:, 5, :], rcp[:])
        nc.vector.scalar_tensor_tensor(                # c_sel - iou
            sel[:, 4, :], iou[:], -1.0, sel[:, 4, :],
            op0=Alu.mult, op1=Alu.add)

        # ---------- masks ----------
        mo = wk.tile([P, F], f32, tag="mo")
        nc.vector.tensor_single_scalar(mo[:], tv[:, 4, :], 0.0, op=Alu.is_gt)
        mob = wk.tile([P, F], bf16, tag="mob")
        nc.vector.tensor_copy(mob[:], mo[:])
        mo5 = wk.tile([P, F], f32, tag="mo5")
        nc.scalar.mul(mo5[:], mo[:], SQRT5)
        mo25 = wk.tile([P, F], f32, tag="mo25")
        nc.scalar.mul(mo25[:], mo[:], 5.0)
        mnh = wk.tile([P, F], f32, tag="mnh")         # sqrt(.5)*(1-mo)
        nc.vector.tensor_scalar(mnh[:], mo[:], -SQH, SQH,
                                op0=Alu.mult, op1=Alu.add)

        nc.vector.tensor_mul(sel[:, 0:2, :], sel[:, 0:2, :], bc(mo5[:], 2))
        nc.vector.tensor_mul(sel[:, 2:4, :], sel[:, 2:4, :], bc(mo25[:], 2))
        nc.vector.tensor_mul(sel[:, 4, :], sel[:, 4, :], mo[:])
        mtwh = wk.tile([P, 2, F], f32, tag="mtwh")
        nc.gpsimd.tensor_mul(mtwh[:], tv[:, 2:4, :], bc(mo25[:], 2))

        nc.scalar.activation(sel[:, 2:4, :], sel[:, 2:4, :], Act.Sqrt,
                             bias=eps5c[:])
        st = wk.tile([P, 2, F], f32, tag="stw")
        nc.scalar.activation(st[:], mtwh[:], Act.Sqrt, bias=eps5c[:])
        nc.vector.tensor_sub(sel[:, 2:4, :], sel[:, 2:4, :], st[:])

        # noobj into dead I/D slots
        nc.vector.tensor_mul(sel[:, 5:7, :], pv[:, 4:10:5, :], bc(mnh[:], 2))

        # ---------- cls: sub on gpsimd (bf16 out), mask on DVE in bf16 2x ----
        mcls = wk.tile([P, 20, F], bf16, tag="mcls")
        nc.gpsimd.tensor_sub(mcls[:], pv[:, 10:30, :], tv[:, 10:30, :])
        nc.vector.tensor_mul(mcls[:], mcls[:], bc(mob[:], 20))

        # ---------- square-accumulate ----------
        nc.scalar.activation(sel[:, 0:7, :], sel[:, 0:7, :], Act.Square,
                             accum_out=stats[:, 2 * t:2 * t + 1])
        nc.scalar.activation(mcls[:], mcls[:], Act.Square,
                             accum_out=stats[:, 2 * t + 1:2 * t + 2])

    total = stp.tile([P, 1], f32)
    nc.vector.tensor_reduce(total[:], stats[:], axis=mybir.AxisListType.X,
                            op=Alu.add)
    nc.sync.dma_start(out_ap, total[:])


def _build():
    if "nc" in _CACHE:
        return _CACHE["nc"]
    nc = bacc.Bacc("TRN2", target_bir_lowering=False, debug=False)
    pred = nc.dram_tensor("predicts", [P, T_TILES, C, F], f32,
                          kind="ExternalInput")
    targ = nc.dram_tensor("targets", [P, T_TILES, C, F], f32,
                          kind="ExternalInput")
    out = nc.dram_tensor("out", [P, 1], f32, kind="ExternalOutput")
    with tile.TileContext(nc) as tc, ExitStack() as ctx:
        _build_body(tc, ctx, pred.ap(), targ.ap(), out.ap())
    nc.compile()
    _CACHE["nc"] = nc
    return nc


def _shard(x):
    # [8192,7,7,30] -> per-core [P, T, C, F] channel-planar
    x = np.ascontiguousarray(x, dtype=np.float32)
    out = []
    for i in range(N_CORES):
        s = x[i * (BATCH // N_CORES):(i + 1) * (BATCH // N_CORES)]
        s = s.reshape(P, T_TILES, F, C).transpose(0, 1, 3, 2)
        out.append(np.ascontiguousarray(s))
    return out


def run(predicts, targets, trace=False, **trace_kwargs):
    nc = _build()
    pshards = _shard(predicts)
    tshards = _shard(targets)
    in_maps = [{"predicts": pshards[i], "targets": tshards[i]}
               for i in range(N_CORES)]
    res = bass_utils.run_bass_kernel_spmd(
        nc, in_maps, core_ids=list(range(N_CORES)), trace=trace,
        **trace_kwargs)
    partial = np.zeros((), dtype=np.float64)
    for r in res.results:
        partial += np.asarray(r["out"], dtype=np.float64).sum()
    return np.float32(partial), res


def kernel(predicts, targets):
    out, _ = run(predicts, targets, trace=False)
    return out


# revision 10
# speedup vs baseline: 1.4032x; 1.2361x over previous
"""YOLOv1 loss (nn_LossModul_16277926052544) on 8 TRN2 NeuronCores.

Pure data parallel: batch 8192 -> 8 shards of 1024. Each core computes a
partial sum of the loss over its shard; host sums the 8x128 partials.

Math restructuring vs the reference (validated to 5.5e-07 rel err in numpy;
bf16 variant 1.9e-04):
  * IoU is translation invariant -> grid offsets cancel; overlap length per
    axis is min(pw, tw, (pw+tw)/2 - |c|) clamped to >= 0, c = (px-tx)/S.
  * resp = iou1 > iou2 <=> inter1*den2 > inter2*den1 (dens > 0).
  * Every loss term is a masked square; weights fold into masks or the
    Scalar engine's Square scale, so ACT Square+accum reduces everything.

Perf structure:
  * channel-planar SBUF tiles ([P, C, F], inner F contiguous) -- DVE pays a
    per-AP-row bubble, so interleaved-channel inner dims are ~8x slower.
  * bf16 intermediates -> DVE 2x_1P packing halves SBUF port traffic (the
    engines contend for the same ports, so total traffic is the limit).
  * DMAs cast f32->bf16 in flight (SWDGE) and are channel-split so the
    geometry pipeline starts after ~1.5MB instead of 6MB; the 5 unused
    target channels (5:10) are never transferred.
"""
import sys

for _p in ("/opt/trn_rl_repo",):
    if _p not in sys.path:
        sys.path.insert(0, _p)

import numpy as np
from contextlib import ExitStack

import concourse.bass as bass  # noqa: F401  (registers engines)
from concourse import bacc, mybir
from concourse import bass_utils
import concourse.tile as tile

N_CORES = 8
BATCH = 8192
S = 7
C = 30
TC = 25                                       # target channels kept: 0:5 + 10:30
P = 128
CELLS_PER_CORE = (BATCH // N_CORES) * S * S   # 50176
F_TOTAL = CELLS_PER_CORE // P                 # 392
T_TILES = 2
F = F_TOTAL // T_TILES                        # 196
R = 1.0 / S
EPS5 = 5e-6                                   # 5 * EPS (lambda folded)
SQRT5 = float(np.sqrt(5.0))
SQH = float(np.sqrt(0.5))

f32 = mybir.dt.float32
bf16 = mybir.dt.bfloat16
u32 = mybir.dt.uint32
Alu = mybir.AluOpType
Act = mybir.ActivationFunctionType

_CACHE = {}


def _build_body(tc, ctx, pred, targ, out_ap):
    nc = tc.nc
    inpool = ctx.enter_context(tc.tile_pool(name="in", bufs=2))
    wk = ctx.enter_context(tc.tile_pool(name="wk", bufs=2))
    stp = ctx.enter_context(tc.tile_pool(name="st", bufs=1))
    stats = stp.tile([P, 3 * T_TILES], f32)
    eps5c = stp.tile([P, 1], f32)               # bias const for Sqrt
    nc.gpsimd.memset(eps5c[:], EPS5)

    for t in range(T_TILES):
        # --- channel-split casting DMAs (SWDGE casts f32->bf16 in flight)
        ptile = inpool.tile([P, C, F], bf16, tag="p")
        ttile = inpool.tile([P, TC, F], bf16, tag="t")
        nc.gpsimd.dma_start(ttile[:, 0:5, :], targ[:, t, 0:5])     # tar box
        nc.gpsimd.dma_start(ptile[:, 0:10, :], pred[:, t, 0:10])   # p boxes
        nc.gpsimd.dma_start(ptile[:, 10:30, :], pred[:, t, 10:30])  # p cls
        nc.gpsimd.dma_start(ttile[:, 5:25, :], targ[:, t, 10:30])  # tar cls

        pv = ptile[:]                                   # [P,30,F] bf16
        tv = ttile[:]                                   # [P,25,F] bf16
        pb = pv[:, 0:10, :].rearrange("p (b c) f -> p b c f", b=2)

        def bc(ap_pf, k):
            a = ap_pf
            if a.ndim == 2:
                a = a.unsqueeze(1)
            return a.broadcast_to([P, k, F])

        flat = lambda a: a.rearrange("p b c f -> p (b c f)")

        # ---------- geometry (bf16) ----------
        dxy = wk.tile([P, 2, 2, F], bf16, tag="dxy")   # [box, xy, f] raw p-t
        nc.gpsimd.tensor_sub(
            dxy[:], pb[:, :, 0:2, :],
            tv[:, 0:2, :].unsqueeze(1).broadcast_to([P, 2, 2, F]))
        sth = wk.tile([P, 2, F], bf16, tag="sth")      # 0.5*twh
        nc.vector.tensor_scalar_mul(sth[:], tv[:, 2:4, :], 0.5)
        s = wk.tile([P, 2, 2, F], bf16, tag="s")       # 0.5*pwh + 0.5*twh
        for b in range(2):                              # STT is <=3D
            nc.vector.scalar_tensor_tensor(
                s[:, b], pb[:, b, 2:4, :], 0.5, sth[:],
                op0=Alu.mult, op1=Alu.add)
        absd = wk.tile([P, 2, 2, F], bf16, tag="absd")  # |dxy|
        nc.scalar.activation(flat(absd[:]), flat(dxy[:]), Act.Abs)
        m = wk.tile([P, 2, 2, F], bf16, tag="m")       # s - R*|dxy|
        nc.vector.scalar_tensor_tensor(
            flat(m[:]), flat(absd[:]), -R, flat(s[:]),
            op0=Alu.mult, op1=Alu.add)
        minwh = wk.tile([P, 2, 2, F], bf16, tag="minwh")
        nc.vector.tensor_tensor(
            minwh[:], pb[:, :, 2:4, :],
            tv[:, 2:4, :].unsqueeze(1).broadcast_to([P, 2, 2, F]), op=Alu.min)
        ln = wk.tile([P, 2, 2, F], bf16, tag="ln")     # overlap lengths
        nc.vector.scalar_tensor_tensor(
            flat(ln[:]), flat(m[:]), 0.0, flat(minwh[:]),
            op0=Alu.max, op1=Alu.min)

        ID = wk.tile([P, 4, F], bf16, tag="ID")        # [I1,I2,D1,D2]
        nc.vector.tensor_mul(ID[:, 0:2, :], ln[:, :, 0, :], ln[:, :, 1, :])
        nc.vector.tensor_mul(ID[:, 2:4, :], pv[:, 2:8:5, :], pv[:, 3:9:5, :])
        tarea = wk.tile([P, 1, F], bf16, tag="tarea")
        nc.vector.tensor_mul(tarea[:], tv[:, 2:3, :], tv[:, 3:4, :])
        nc.gpsimd.tensor_sub(ID[:, 2:4, :], ID[:, 2:4, :], ID[:, 0:2, :])
        nc.gpsimd.tensor_add(ID[:, 2:4, :], ID[:, 2:4, :], bc(tarea[:], 2))

        g = wk.tile([P, 2, F], f32, tag="g")
        nc.vector.tensor_mul(g[:, 0, :], ID[:, 0, :], ID[:, 3, :])
        nc.vector.tensor_mul(g[:, 1, :], ID[:, 1, :], ID[:, 2, :])
        resp = wk.tile([P, F], u32, tag="resp")        # 1 -> box1
        nc.vector.tensor_tensor(resp[:], g[:, 0, :], g[:, 1, :], op=Alu.is_gt)

        # ---------- selects (box2 copied, box1 predicated over it) ----------
        sel = wk.tile([P, 7, F], bf16, tag="sel")      # dx dy w h c n1 n2
        nc.scalar.copy(sel[:, 0:2, :], dxy[:, 1, :, :])
        nc.scalar.copy(sel[:, 2:5, :], pv[:, 7:10, :])
        nc.vector.copy_predicated(sel[:, 0:2, :], bc(resp[:], 2), dxy[:, 0, :, :])
        nc.vector.copy_predicated(sel[:, 2:5, :], bc(resp[:], 3), pv[:, 2:5, :])
        idsel = wk.tile([P, 2, F], f32, tag="idsel")   # [Isel, Dsel] f32
        nc.scalar.copy(idsel[:], ID[:, 1:4:2, :])
        nc.vector.copy_predicated(idsel[:], bc(resp[:], 2), ID[:, 0:3:2, :])

        rcp = wk.tile([P, F], f32, tag="rcp")
        nc.vector.reciprocal_approx_fast(rcp[:], idsel[:, 1, :])
        iou = wk.tile([P, F], f32, tag="iou")
        nc.vector.tensor_mul(iou[:], idsel[:, 0, :], rcp[:])
        nc.vector.scalar_tensor_tensor(                 # c_sel - iou
            sel[:, 4, :], iou[:], -1.0, sel[:, 4, :],
            op0=Alu.mult, op1=Alu.add)

        # ---------- masks (bf16; weights exact: 5 in bf16, sqrt5 via ACT) ---
        mo = wk.tile([P, F], bf16, tag="mo")
        nc.vector.tensor_single_scalar(mo[:], tv[:, 4, :], 0.0, op=Alu.is_gt)
        mo25 = wk.tile([P, F], bf16, tag="mo25")
        nc.scalar.mul(mo25[:], mo[:], 5.0)
        mnh = wk.tile([P, F], bf16, tag="mnh")          # sqrt(.5)*(1-mo)
        nc.vector.tensor_scalar(mnh[:], mo[:], -SQH, SQH,
                                op0=Alu.mult, op1=Alu.add)

        nc.vector.tensor_mul(sel[:, 0:2, :], sel[:, 0:2, :], bc(mo[:], 2))
        nc.vector.tensor_mul(sel[:, 2:4, :], sel[:, 2:4, :], bc(mo25[:], 2))
        nc.vector.tensor_mul(sel[:, 4, :], sel[:, 4, :], mo[:])
        mtwh = wk.tile([P, 2, F], bf16, tag="mtwh")
        nc.gpsimd.tensor_mul(mtwh[:], tv[:, 2:4, :], bc(mo25[:], 2))

        nc.scalar.activation(sel[:, 2:4, :], sel[:, 2:4, :], Act.Sqrt,
                             bias=eps5c[:])
        st = wk.tile([P, 2, F], bf16, tag="stw")
        nc.scalar.activation(st[:], mtwh[:], Act.Sqrt, bias=eps5c[:])
        nc.vector.tensor_sub(sel[:, 2:4, :], sel[:, 2:4, :], st[:])

        # noobj conf (slots 5,6)
        nc.vector.tensor_mul(sel[:, 5:7, :], pv[:, 4:10:5, :], bc(mnh[:], 2))

        # ---------- cls ----------
        mcls = wk.tile([P, 20, F], bf16, tag="mcls")
        nc.gpsimd.tensor_sub(mcls[:], pv[:, 10:30, :], tv[:, 5:25, :])
        nc.vector.tensor_mul(mcls[:], mcls[:], bc(mo[:], 20))

        # ---------- square-accumulate (3 slots: xy*5, rest, cls) ----------
        nc.scalar.activation(sel[:, 0:2, :], sel[:, 0:2, :], Act.Square,
                             scale=SQRT5, accum_out=stats[:, 3 * t:3 * t + 1])
        nc.scalar.activation(sel[:, 2:7, :], sel[:, 2:7, :], Act.Square,
                             accum_out=stats[:, 3 * t + 1:3 * t + 2])
        nc.scalar.activation(mcls[:], mcls[:], Act.Square,
                             accum_out=stats[:, 3 * t + 2:3 * t + 3])

    total = stp.tile([P, 1], f32)
    nc.vector.tensor_reduce(total[:], stats[:], axis=mybir.AxisListType.X,
                            op=Alu.add)
    nc.sync.dma_start(out_ap, total[:])


def _build():
    if "nc" in _CACHE:
        return _CACHE["nc"]
    nc = bacc.Bacc("TRN2", target_bir_lowering=False, debug=False)
    pred = nc.dram_tensor("predicts", [P, T_TILES, C, F], f32,
                          kind="ExternalInput")
    targ = nc.dram_tensor("targets", [P, T_TILES, C, F], f32,
                          kind="ExternalInput")
    out = nc.dram_tensor("out", [P, 1], f32, kind="ExternalOutput")
    with tile.TileContext(nc) as tc, ExitStack() as ctx:
        _build_body(tc, ctx, pred.ap(), targ.ap(), out.ap())
    nc.compile()
    _CACHE["nc"] = nc
    return nc


def _shard(x):
    # [8192,7,7,30] -> per-core [P, T, C, F] channel-planar
    x = np.ascontiguousarray(x, dtype=np.float32)
    out = []
    for i in range(N_CORES):
        s = x[i * (BATCH // N_CORES):(i + 1) * (BATCH // N_CORES)]
        s = s.reshape(P, T_TILES, F, C).transpose(0, 1, 3, 2)
        out.append(np.ascontiguousarray(s))
    return out


def run(predicts, targets, trace=False, **trace_kwargs):
    nc = _build()
    pshards = _shard(predicts)
    tshards = _shard(targets)
    in_maps = [{"predicts": pshards[i], "targets": tshards[i]}
               for i in range(N_CORES)]
    res = bass_utils.run_bass_kernel_spmd(
        nc, in_maps, core_ids=list(range(N_CORES)), trace=trace,
        **trace_kwargs)
    partial = np.zeros((), dtype=np.float64)
    for r in res.results:
        partial += np.asarray(r["out"], dtype=np.float64).sum()
    return np.float32(partial), res


def kernel(predicts, targets):
    out, _ = run(predicts, targets, trace=False)
    return out
